# revision 27
# baseline (speedup 1.0000x reference)
"""ConvDownsample2D (StyleGAN2 FIR blur + strided conv) for 8 Trainium2 cores.

Sharding: data-parallel over batch, 1 image per NeuronCore.

Per-core pipeline (all compute in fp16 with fp32 PSUM accumulation):
  1. x is passed untransposed as [H, W, C] fp16 (host does only a dtype
     cast). The separable blur is symmetric, so the device blurs the H
     axis on the TensorEngine and the W axis on DVE/GpSimd; conv weight
     taps are transposed on host to match, and the [OW, OH, OC] fp16
     device output is untransposed + upcast on host.
  2. Axis-0 blur runs ON THE TENSOR ENGINE as a banded matmul
     y_h = x_tile.T @ B (contraction over image rows) which also
     transposes NHWC into channel-major layout for free. PSUM pairs are
     evacuated to SBUF by the Activation engine.
  3. Axis-1 blur runs as a cascade of 2-tap adds ([1,3,3,1] = [1,1]^3),
     column-split between DVE and GpSimd (GpSimd TT is ~3.9x slower, so
     it gets ~30% of the width) and row-chunked (NCHUNK) so early conv
     groups can start before the whole block is blurred.
  4. The 3x3/stride-2 conv is 9 accumulating matmuls per output row
     (lhsT = blurred activations [C,128pix], rhs = W taps [C,256oc]).
     PSUM banks hold 2 output rows; evacuation is a DVE tensor_add that
     also applies the conv bias and casts to the fp16 output (no bias
     matmul on the PE).
  Emission is software-pipelined: the axis-0-blur pairs of block k are
  front-biased and interleaved with the conv groups of block k-1 (RPC
  pairs per conv group) so the PE never stalls on PSUM evacuation; input
  DMAs split the two image halves across the SP and Activation HWDGE
  rings.
"""
import sys

if "/opt/trn_rl_repo" not in sys.path:
    sys.path.insert(0, "/opt/trn_rl_repo")

import numpy as np

import concourse.bass as bass
import concourse.tile as tile
from concourse import bacc, mybir
from concourse.bass_utils import run_bass_kernel_spmd

F16 = mybir.dt.float16
F32 = mybir.dt.float32

N_CORES = 8
H = W = 256
C = 128
OC = 256
OH = OW = 128
WP = W + 1          # 257 blurred width
PITCH = 258         # even row pitch (fp16 4B alignment for DVE 2x mode)
import os as _os
XGRP = int(_os.environ.get("KXGRP", "18"))   # y_h rows loaded per input DMA
SPLIT = int(_os.environ.get("KSPLIT", "182"))  # DVE v-blur columns [0:SPLIT)
PYH_BUFS = int(_os.environ.get("KPYH", "2"))
POUT_BUFS = int(_os.environ.get("KPOUT", "4"))
RPC = int(_os.environ.get("KRPC", "4"))  # pairs emitted per interleaved conv sub
NCHUNK = int(_os.environ.get("KCHUNK", "6"))  # v-blur row chunks per block
_sizes = [int(v) for v in _os.environ.get(
    "KBLOCKS", "8,16,16,16,16,16,16,16,8").split(",")]
BLOCKS = []
_p = 0
for _s in _sizes:
    BLOCKS.append((_p, _s))
    _p += _s
assert _p == OH
N_YH = 2 * max(_sizes) + 4   # max y_h rows per block incl halo
N_YV = 2 * max(_sizes) + 1   # max y_v rows per block


def _build_bass(mode, repeat=1):
    nc = bacc.Bacc("TRN2", target_bir_lowering=False, debug=False)

    x16 = nc.dram_tensor("x16", [H, W, C], F16, kind="ExternalInput").ap()
    b_a = nc.dram_tensor("b_a", [128, 131], F16, kind="ExternalInput").ap()
    b_b = nc.dram_tensor("b_b", [128, 130], F16, kind="ExternalInput").ap()
    w16 = nc.dram_tensor("w16", [9, C, OC], F16, kind="ExternalInput").ap()
    biasb = nc.dram_tensor("biasb", [128, 2, OC], F32, kind="ExternalInput").ap()
    if mode == "general":
        kvt = nc.dram_tensor("kvt", [128, 4], F32, kind="ExternalInput").ap()
    out = nc.dram_tensor("out", [OH, OW, OC], F16, kind="ExternalOutput").ap()

    if mode == "b1331":
        stage_shifts = [1, 1, 1]
    elif mode == "b1111":
        stage_shifts = [1, 2]
    else:
        stage_shifts = None

    with tile.TileContext(nc) as tc:
        with (
            tc.tile_pool(name="const", bufs=1) as cpool,
            tc.tile_pool(name="xin", bufs=int(_os.environ.get("KXBUFS", "10"))) as xpool,
            tc.tile_pool(name="yh", bufs=int(_os.environ.get("KYH", "3"))) as yhpool,
            tc.tile_pool(name="yv", bufs=int(_os.environ.get("KYV", "2"))) as yvpool,
            tc.tile_pool(name="osb", bufs=int(_os.environ.get("KOSB", "3"))) as opool,
            tc.tile_pool(name="pyh", bufs=PYH_BUFS, space=bass.MemorySpace.PSUM) as pyh,
            tc.tile_pool(name="pout", bufs=POUT_BUFS, space=bass.MemorySpace.PSUM) as pout,
        ):
            ba_sb = cpool.tile([128, 131], F16)
            nc.scalar.dma_start(ba_sb[:], b_a[:])
            bb_sb = cpool.tile([128, 130], F16)
            nc.scalar.dma_start(bb_sb[:], b_b[:])
            w_sb = cpool.tile([128, 9, OC], F16)
            bias_sb = cpool.tile([128, 2, OC], F32)
            if mode == "general":
                kv_sb = cpool.tile([128, 4], F32, tag="kv")
            else:
                kv_sb = None

            def load_weights():
                # deferred so early input DMAs go first on the DGE queues
                nc.sync.dma_start(w_sb[:], w16.rearrange("t c o -> c t o"))
                nc.sync.dma_start(bias_sb[:], biasb[:])
                if mode == "general":
                    nc.sync.dma_start(kv_sb[:], kvt[:])

            def emit_loads(k, rep):
                """Input DMAs + edge memsets for block k. Returns
                (yh_t, pair-emitter list) for the axis-0 blur."""
                p0, pblk = BLOCKS[k]
                n_yh = 2 * pblk + 4
                hs0 = 2 * p0 - 2          # first y_h row (may be <0 / >=W)
                s_lo = max(0, -hs0)       # first valid slot
                s_hi = min(n_yh, W - hs0)  # end of valid slots (both even)

                yh_t = yhpool.tile([128, N_YH, PITCH], F16, tag="yh")
                if s_lo > 0:
                    nc.gpsimd.memset(yh_t[:, 0:s_lo, :], 0.0)
                if s_hi < n_yh:
                    nc.gpsimd.memset(yh_t[:, s_hi:n_yh, :], 0.0)

                # ---- input loads: XGRP y_h rows per DMA, 2 halves ----
                if k <= 1:  # fine first loads: PE starts sooner
                    bounds = [s_lo, s_lo + 2, s_lo + 9] if k == 0 else [s_lo]
                    while bounds[-1] < s_hi:
                        bounds.append(min(bounds[-1] + 9, s_hi))
                    groups = [
                        (bounds[i], bounds[i + 1] - bounds[i])
                        for i in range(len(bounds) - 1)
                        if bounds[i + 1] > bounds[i]
                    ]
                else:
                    groups = [
                        (g0, min(XGRP, s_hi - g0))
                        for g0 in range(s_lo, s_hi, XGRP)
                    ]
                xtiles = []
                for gi, (g0, sz) in enumerate(groups):
                    pair = []
                    for t in range(2):
                        xt = xpool.tile([128, XGRP, C], F16, tag=f"x{t}")
                        eng = nc.sync if t == 0 else nc.scalar
                        eng.dma_start(
                            xt[:, 0:sz, :],
                            x16[t * 128 : (t + 1) * 128, hs0 + g0 : hs0 + g0 + sz, :],
                        )
                        pair.append(xt)
                    xtiles.append((g0, pair, sz))
                return yh_t, xtiles, s_lo, s_hi

            def pair_emitters(k, yh_t, xtiles, s_lo, s_hi):
                """One closure per axis-0-blur row pair (PE matmuls + ACT
                evacuation)."""

                def xslice(s):
                    for g0, pair, sz in xtiles:
                        if g0 <= s < g0 + sz:
                            return pair[0][:, s - g0, :], pair[1][:, s - g0, :]
                    raise AssertionError(s)

                def emit_pair(s0):
                    pp = pyh.tile([128, 2, 512], F32)
                    for e in range(2):
                        xlo, xhi = xslice(s0 + e)
                        nc.tensor.matmul(
                            pp[:, e, 0:131], xlo, ba_sb[:],
                            start=True, stop=False,
                        )
                        nc.tensor.matmul(
                            pp[:, e, 127:257], xhi, bb_sb[:],
                            start=False, stop=True, skip_group_check=True,
                        )
                    nc.scalar.copy(yh_t[:, s0 : s0 + 2, 0:WP], pp[:, :, 0:WP])

                return [
                    (lambda s0=s0: emit_pair(s0))
                    for s0 in range(s_lo, s_hi, 2)
                ]

            def vblur_chunk(yh_t, yv_t, n_yh, m0, m1):
                """[1,3,3,1] cascade for y_v rows [m0, m1): three in-place
                2-tap passes, each column-split DVE / GpSimd. Pass i covers
                rows [m0+(2-i) if m0 else 0, m1+(2-i) capped). Splitting at
                staggered boundaries keeps chunk writes disjoint."""
                for i in range(3):
                    lo = 0 if m0 == 0 else m0 + (2 - i)
                    hi = min(m1 + (2 - i), n_yh - 1 - i)
                    n = hi - lo
                    if n <= 0:
                        continue
                    dst = yv_t if i == 2 else yh_t
                    if SPLIT < PITCH:
                        nc.gpsimd.tensor_add(
                            dst[:, lo:hi, SPLIT:PITCH],
                            yh_t[:, lo:hi, SPLIT:PITCH],
                            yh_t[:, lo + 1 : hi + 1, SPLIT:PITCH],
                        )
                    nc.vector.tensor_add(
                        dst[:, lo:hi, 0:SPLIT],
                        yh_t[:, lo:hi, 0:SPLIT],
                        yh_t[:, lo + 1 : hi + 1, 0:SPLIT],
                    )

            def vblur(k, yh_t, chunk=None):
                """Axis-1 blur cascade, column-split DVE / GpSimd.
                chunk=(yv_t, m0, m1) emits one b1331 sub-range; chunk=None
                emits the whole block (and allocates yv_t)."""
                p0, pblk = BLOCKS[k]
                n_yh = 2 * pblk + 4
                n_yv = 2 * pblk + 1
                if chunk is not None:
                    yv_t, m0, m1 = chunk
                    vblur_chunk(yh_t, yv_t, n_yh, m0, m1)
                    return yv_t
                yv_t = yvpool.tile([128, N_YV, PITCH], F16, tag="yv")
                if stage_shifts == [1, 1, 1]:
                    vblur_chunk(yh_t, yv_t, n_yh, 0, n_yv)
                elif stage_shifts is not None:
                    n = n_yh
                    for i, sh in enumerate(stage_shifts):
                        n -= sh
                        dst = yv_t if i == len(stage_shifts) - 1 else yh_t
                        nc.gpsimd.tensor_add(
                            dst[:, 0:n, SPLIT:PITCH],
                            yh_t[:, 0:n, SPLIT:PITCH],
                            yh_t[:, sh : sh + n, SPLIT:PITCH],
                        )
                        nc.vector.tensor_add(
                            dst[:, 0:n, 0:SPLIT],
                            yh_t[:, 0:n, 0:SPLIT],
                            yh_t[:, sh : sh + n, 0:SPLIT],
                        )
                    assert n == n_yv
                else:
                    nc.gpsimd.tensor_scalar(
                        yv_t[:, 0:n_yv, SPLIT:PITCH],
                        yh_t[:, 0:n_yv, SPLIT:PITCH],
                        kv_sb[:, 0:1],
                        None,
                        mybir.AluOpType.mult,
                    )
                    nc.vector.tensor_scalar(
                        yv_t[:, 0:n_yv, 0:SPLIT],
                        yh_t[:, 0:n_yv, 0:SPLIT],
                        kv_sb[:, 0:1],
                        None,
                        mybir.AluOpType.mult,
                    )
                    for u in range(1, 4):
                        nc.gpsimd.scalar_tensor_tensor(
                            yv_t[:, 0:n_yv, SPLIT:PITCH],
                            yh_t[:, u : u + n_yv, SPLIT:PITCH],
                            kv_sb[:, u : u + 1],
                            yv_t[:, 0:n_yv, SPLIT:PITCH],
                            mybir.AluOpType.mult,
                            mybir.AluOpType.add,
                        )
                        nc.vector.scalar_tensor_tensor(
                            yv_t[:, 0:n_yv, 0:SPLIT],
                            yh_t[:, u : u + n_yv, 0:SPLIT],
                            kv_sb[:, u : u + 1],
                            yv_t[:, 0:n_yv, 0:SPLIT],
                            mybir.AluOpType.mult,
                            mybir.AluOpType.add,
                        )
                return yv_t

            def conv_emitters(k, yv_t):
                """One closure per 2-output-row conv group (18 PE matmuls
                + DVE bias evac); store DMA every 4 rows."""
                p0, pblk = BLOCKS[k]
                state = {}

                def emit_sub(j):
                    if j % 2 == 0:
                        ot = opool.tile([128, 4, OC], F16, tag="ot")
                        state["ot"] = ot
                    ot = state["ot"]
                    po = pout.tile([128, 2, OC], F32)  # one bank: 2 rows
                    for e in range(2):
                        r0 = 2 * (2 * j + e)
                        for t in range(9):
                            a, b = divmod(t, 3)
                            lhsT = yv_t[:, r0 + a, b : b + 256 : 2]
                            nc.tensor.matmul(
                                po[:, e, :], lhsT, w_sb[:, t, :],
                                start=(e == 0 and t == 0),
                                stop=(e == 1 and t == 8),
                                skip_group_check=True,
                            )
                    # evacuate + bias + fp16 cast in one DVE op
                    sub = j % 2
                    nc.vector.tensor_add(
                        ot[:, 2 * sub : 2 * sub + 2, :], po[:], bias_sb[:]
                    )
                    if sub == 1:
                        p = p0 + 4 * (j // 2)
                        dst = out[p : p + 4, :, :].rearrange("r q o -> q r o")
                        nc.scalar.dma_start(dst, ot[:])

                return [(lambda j=j: emit_sub(j)) for j in range(pblk // 2)]

            # software pipeline: axis0-blur pairs of block k are emitted
            # front-biased (RPC pairs per conv sub of block k-1 filling PE
            # while ACT drains the PSUM pairs); the v-blur runs in NCHUNK
            # row chunks so conv of block k can start before the whole
            # block is blurred.
            for rep in range(repeat):
                prev_conv = []
                for k in range(len(BLOCKS)):
                    yh_t, xtiles, s_lo, s_hi = emit_loads(k, rep)
                    if k == 1 and rep == 0:
                        load_weights()
                    pairs = pair_emitters(k, yh_t, xtiles, s_lo, s_hi)
                    p0, pblk = BLOCKS[k]
                    n_yv = 2 * pblk + 1
                    chunked = stage_shifts == [1, 1, 1]
                    if chunked:
                        yv_t = yvpool.tile([128, N_YV, PITCH], F16, tag="yv")
                        # chunk boundaries (even rows), last chunk to n_yv
                        bnds = [
                            min((n_yv * (i + 1) // NCHUNK) & ~1, n_yv)
                            for i in range(NCHUNK - 1)
                        ] + [n_yv]
                        # pair index after which chunk i's inputs are ready
                        jdone = [(b + 2 - s_lo + 1) // 2 for b in bnds]
                        chunks = list(zip([0] + bnds[:-1], bnds))
                    ci = 0
                    ch = 0
                    for j, fn in enumerate(pairs):
                        fn()
                        while chunked and ch < len(chunks) - 1 and j >= jdone[ch]:
                            vblur(k, yh_t, chunk=(yv_t, *chunks[ch]))
                            ch += 1
                        if (j + 1) % RPC == 0 and ci < len(prev_conv):
                            prev_conv[ci]()
                            ci += 1
                    if chunked:
                        for m0, m1 in chunks[ch:]:
                            vblur(k, yh_t, chunk=(yv_t, m0, m1))
                    else:
                        yv_t = vblur(k, yh_t)
                    while ci < len(prev_conv):
                        prev_conv[ci]()
                        ci += 1
                    prev_conv = conv_emitters(k, yv_t)
                for fn in prev_conv:
                    fn()

    nc.compile()
    return nc


_NC = {}


def _get_nc(mode="b1331", repeat=1):
    key = (mode, repeat)
    if key not in _NC:
        _NC[key] = _build_bass(mode, repeat)
    return _NC[key]


def _blur_mode(bk):
    k8 = bk / bk.sum() * 8.0
    if np.allclose(k8, [1.0, 3.0, 3.0, 1.0], rtol=1e-6, atol=1e-7):
        return "b1331"
    k4 = bk / bk.sum() * 4.0
    if np.allclose(k4, [1.0, 1.0, 1.0, 1.0], rtol=1e-6, atol=1e-7):
        return "b1111"
    return "general"


def _prepare_in_maps(x, conv_w, conv_b, blur_kernel):
    x = np.asarray(x, dtype=np.float32)
    conv_w = np.asarray(conv_w, dtype=np.float32)
    conv_b = np.asarray(conv_b, dtype=np.float32)
    bk = np.asarray(blur_kernel, dtype=np.float32)

    mode = _blur_mode(bk)
    k1 = (bk / bk.sum()).astype(np.float32)  # separable normalized taps

    # banded axis-0 blur matrices (normalization folded in)
    Bfull = np.zeros((W, WP), np.float32)
    j = np.arange(W)[:, None]
    wp = np.arange(WP)[None, :]
    d = j - wp + 2
    m = (d >= 0) & (d <= 3)
    Bfull[m] = k1[d[m]]
    b_a = Bfull[0:128, 0:131].astype(np.float16)
    b_b = Bfull[128:256, 127:257].astype(np.float16)

    # axis-1 normalization: box cascades compute the UNNORMALIZED sum,
    # so fold 1/sum(bk) into the conv weights for those modes.
    wscale = 1.0 / bk.sum() if mode in ("b1331", "b1111") else 1.0
    # device blurs axis0=H on PE and axis1=W on DVE, and the conv walks
    # device rows = W: swap the spatial taps of conv_w.
    w16 = np.ascontiguousarray(
        (conv_w.transpose(1, 0, 2, 3) * wscale).reshape(9, C, OC).astype(np.float16)
    )
    biasb = np.ascontiguousarray(
        np.broadcast_to(conv_b[None, None, :], (128, 2, OC))
    ).astype(np.float32)

    in_maps = []
    for i in range(N_CORES):
        im = {
            "x16": np.ascontiguousarray(x[i].astype(np.float16)),
            "b_a": b_a,
            "b_b": b_b,
            "w16": w16,
            "biasb": biasb,
        }
        if mode == "general":
            im["kvt"] = np.ascontiguousarray(
                np.broadcast_to(k1[None, :], (128, 4)).astype(np.float32)
            )
        in_maps.append(im)
    return mode, in_maps


def _run(mode, in_maps, **kwargs):
    nc = _get_nc(mode)
    return run_bass_kernel_spmd(nc, in_maps, core_ids=list(range(N_CORES)), **kwargs)


def kernel(x, conv_w, conv_b, blur_kernel):
    mode, in_maps = _prepare_in_maps(x, conv_w, conv_b, blur_kernel)
    res = _run(mode, in_maps)
    # device output is [OW, OH, OC] fp16; untranspose + fp32 on host
    return np.stack(
        [
            np.asarray(res.results[i]["out"]).transpose(1, 0, 2).astype(np.float32)
            for i in range(N_CORES)
        ],
        axis=0,
    )

